# revision 17
# baseline (speedup 1.0000x reference)
"""Trainium2 Bass kernel for nn_ClassAtt (dense MLP + 3-way class attention).

Model (per row of tube [B, 1536]):
  x1,x2,x3 = tube split into 3x512
  P_i   = relu(x_i @ w_i.T + b_i)            [B, 1024]
  last  = relu(concat(P1,P2,P3) @ wh.T + bh) [B, 1024]
  a_i   = rowwise_dot(last, P_i); w = softmax(a)  [B, 3]
  ctx   = sum_i w_i * P_i                    [B, 1024]
  out   = relu(concat(ctx, last) @ wd1.T + bd1) @ wd2.T + bd2  [B, 1000]

Strategy (v5): pure data parallel over 8 NeuronCores (2048 rows each).
All matmuls in bf16 (full PE rate, half the DMA/SBUF of fp32) with fp32
PSUM accumulation; rel err vs fp32 reference ~5.5e-3.  Activations in
transposed [feature, row] layout: contraction on SBUF partitions,
biases per-partition scalars.

Two fused phases, one DRAM spill (dec=[ctx;last]) between them; row
tiles of 256 in both.  A 10-matmul fp32 warmup at t=0 keeps the PE HAM
clock-gate warm through the initial weight load.

DMA: SDMA engines round-robin *packet slots* across active queues, so
per-queue bandwidth ~ packet size.  Every DRAM tensor is therefore
host-prearranged into its exact SBUF layout (partition-major) so each
transfer is a single DMA with multi-KB per-partition contiguity:
  xT [8,128,12,256] (one 6KB/part DMA per row tile, sync ring)
  w1/w2/w3 [128,4,1024], wh [128,24,1024] (scalar ring; wh in 4 groups
    so L2 tile 0 starts as chunks land)
  wd1 [128,16,2048] (gpsimd SWDGE, 3 chunks per row tile from tile 1 —
    off the scalar ring so L2's FIFO-count wait never covers it, and
    spread so it doesn't steal packet slots from wh)
  dec spill [8,128,16,256] A-tile-major (8KB/part contiguous both ways)
  out writes alternate sync/gpsimd to halve the final drain.
Phase-B pools open pbd first so dec loads land on the SBUF region
freed earliest by phase A (shortest WAR wait at the transition).
"""

import numpy as np
import ml_dtypes

import concourse.bass as bass
import concourse.mybir as mybir
import concourse.tile as tile
from concourse import bacc
from concourse.bass_utils import run_bass_kernel_spmd

F32 = mybir.dt.float32
F32R = mybir.dt.float32r
BF16 = mybir.dt.bfloat16

N_CORES = 8
B = 16384
ROWS = B // N_CORES  # rows per core
M = 1024             # hidden width
DEC_H = 2048
OUT = 1000
R1 = 256
NT1 = ROWS // R1

AluOp = mybir.AluOpType
Act = mybir.ActivationFunctionType


def build_nc():
    nc = bacc.Bacc(None, target_bir_lowering=False)

    # ---- DRAM I/O (per-core shapes, partition-major SBUF layouts) ----
    xT = nc.dram_tensor("xT", [NT1, 128, 12, R1], BF16, kind="ExternalInput")
    wT = [
        nc.dram_tensor(f"w{i + 1}T", [128, 4, M], BF16, kind="ExternalInput")
        for i in range(3)
    ]
    whT = nc.dram_tensor("whT", [128, 24, M], BF16, kind="ExternalInput")
    wd1T = nc.dram_tensor("wd1T", [128, 16, DEC_H], BF16, kind="ExternalInput")
    wd2T = nc.dram_tensor("wd2T", [128, 16, OUT], BF16, kind="ExternalInput")
    bv = [
        nc.dram_tensor(f"b{i + 1}", [128, 8], F32, kind="ExternalInput")
        for i in range(3)
    ]
    bh = nc.dram_tensor("bh", [128, 8], F32, kind="ExternalInput")
    bd1 = nc.dram_tensor("bd1", [128, 16], F32, kind="ExternalInput")
    bd2 = nc.dram_tensor("bd2", [128, 8], F32, kind="ExternalInput")
    outT = nc.dram_tensor("outT", [OUT, ROWS], F32, kind="ExternalOutput")

    with tile.TileContext(nc) as tc:
        with tc.tile_pool(name="dram", bufs=1, space="DRAM") as dram:
            dec = dram.tile([NT1, 128, 16, R1], BF16)  # [ctx; last]

            # Outer pool: survives both phases (wd1 streams in during A).
            with tc.tile_pool(name="pw", bufs=1) as pw:
                wd1_sb = pw.tile([128, 16, DEC_H], BF16)
                bd1_sb = pw.tile([128, 16], F32, tag="bd1")
                bd2_sb = pw.tile([128, 8], F32, tag="bd2")
                ones_f32 = pw.tile([128, 128], F32, tag="ones_f32")
                ones_sb = pw.tile([128, 128], BF16, tag="ones")

                # ================= Phase A =================
                with (
                    tc.tile_pool(name="paw", bufs=1) as paw,
                    tc.tile_pool(name="pax", bufs=2) as pax,
                    tc.tile_pool(name="pap", bufs=2) as pap,
                    tc.tile_pool(name="pad", bufs=2) as pad,
                    tc.tile_pool(name="pat", bufs=1) as pat,
                    tc.tile_pool(name="pas", bufs=1) as pas,
                    tc.tile_pool(name="psA", bufs=4, space="PSUM") as psA,
                    tc.tile_pool(name="psAl", bufs=3, space="PSUM") as psAl,
                ):
                    xts = {}

                    def load_xt(rt):
                        t = pax.tile([128, 12, R1], BF16, tag="xt", name="xt")
                        nc.sync.dma_start(t, xT.ap()[rt])
                        xts[rt] = t

                    nc.any.memset(ones_f32, 1.0)
                    nc.vector.tensor_copy(ones_sb, ones_f32)
                    # PE warmup: keep the HAM clock-gate busy while the
                    # first weights stream in (fp32: 4 cyc/row).
                    for _ in range(20):
                        wps = psA.tile([128, R1], F32, tag="mm", name="warm")
                        nc.tensor.matmul(wps[:, 0:128], ones_f32, ones_f32,
                                         start=True, stop=True)

                    # scalar ring: w1..w3, bh, wh (4 groups) — nothing
                    # else, so L2's FIFO-count wait covers only these.
                    w_sb = []
                    b_sb = []
                    for i in range(3):
                        w = paw.tile([128, 4, M], BF16, tag=f"w{i}",
                                     name=f"w{i}")
                        nc.scalar.dma_start(w, wT[i].ap())
                        b = paw.tile([128, 8], F32, tag=f"b{i}", name=f"b{i}")
                        nc.scalar.dma_start(b, bv[i].ap())
                        w_sb.append(w)
                        b_sb.append(b)
                        if i == 0:
                            load_xt(0)
                        if i == 1:
                            load_xt(1)
                    bh_sb = paw.tile([128, 8], F32, tag="bh", name="bh")
                    nc.scalar.dma_start(bh_sb, bh.ap())
                    wh_sb = paw.tile([128, 24, M], BF16, tag="wh", name="wh")
                    # wh split across BOTH HWDGE rings: scalar has w1-w3
                    # (3.15MB) queued, sync only xt0/xt1 (1.6MB) — halves
                    # the arrival time of the full 6.3MB for L2 tile 0.
                    for g in range(4):
                        eng = nc.scalar if g % 2 == 0 else nc.sync
                        eng.dma_start(wh_sb[:, 6 * g:6 * g + 6, :],
                                      whT.ap()[:, 6 * g:6 * g + 6, :])
                    nc.gpsimd.dma_start(bd1_sb, bd1.ap())
                    nc.gpsimd.dma_start(bd2_sb, bd2.ap())

                    for rt in range(NT1):
                        xt = xts.pop(rt)

                        # ---- L1: P_i = relu(x_i @ w_i.T + b_i) ----
                        pt = []
                        for i in range(3):
                            p_i = pap.tile([128, 8, R1], BF16, tag=f"p{i}",
                                           name=f"p{i}")
                            for fc in range(8):
                                ps = psA.tile([128, R1], F32, tag="mm",
                                              name="ps1")
                                for kc in range(4):
                                    nc.tensor.matmul(
                                        ps,
                                        w_sb[i][:, kc, fc * 128:(fc + 1) * 128],
                                        xt[:, i * 4 + kc, :],
                                        start=(kc == 0),
                                        stop=(kc == 3),
                                    )
                                nc.scalar.activation(
                                    p_i[:, fc, :], ps, Act.Relu,
                                    bias=b_sb[i][:, fc:fc + 1],
                                )
                            pt.append(p_i)

                        # wd1 stream, spread so it doesn't contend with wh
                        # for SDMA packet slots early on (gpsimd ring).
                        if 1 <= rt <= 5:
                            for kc in range(3 * (rt - 1), 3 * rt):
                                nc.gpsimd.dma_start(wd1_sb[:, kc, :],
                                                    wd1T.ap()[:, kc, :])
                        if rt == 6:
                            nc.gpsimd.dma_start(wd1_sb[:, 15, :],
                                                wd1T.ap()[:, 15, :])

                        # ---- L2: last = relu(hid1 @ wh.T + bh) ----
                        dec_sb = pad.tile([128, 16, R1], BF16, tag="dec",
                                          name="dec")
                        last = dec_sb[:, 8:16, :]
                        for fc in range(8):
                            ps = psA.tile([128, R1], F32, tag="mm", name="ps2")
                            for i in range(3):
                                for kc in range(8):
                                    nc.tensor.matmul(
                                        ps,
                                        wh_sb[:, i * 8 + kc,
                                              fc * 128:(fc + 1) * 128],
                                        pt[i][:, kc, :],
                                        start=(i == 0 and kc == 0),
                                        stop=(i == 2 and kc == 7),
                                    )
                            nc.scalar.activation(
                                last[:, fc, :], ps, Act.Relu,
                                bias=bh_sb[:, fc:fc + 1],
                            )

                        # ---- attention: alphas via bf16 ones-matmul ----
                        aps = []
                        for i in range(3):
                            tmp = pat.tile([128, 8, R1], BF16, tag="tmp",
                                           name=f"tmp{i}")
                            nc.vector.tensor_tensor(tmp, last, pt[i],
                                                    AluOp.mult)
                            ap_i = psAl.tile([128, R1], F32, tag="alpha",
                                             name=f"alpha{i}")
                            for fc in range(8):
                                nc.tensor.matmul(
                                    ap_i, ones_sb, tmp[:, fc, :],
                                    start=(fc == 0), stop=(fc == 7),
                                )
                            aps.append(ap_i)

                        # softmax over the 3 logits (fp32)
                        asb = pas.tile([128, 3, R1], F32, tag="asb")
                        for i in range(3):
                            nc.scalar.copy(asb[:, i, :], aps[i])
                        ai = asb.rearrange("p i r -> p r i")
                        mx = pas.tile([128, R1], F32, tag="mx")
                        nc.vector.reduce_max(mx, ai, axis=mybir.AxisListType.X)
                        bshp = (128, 3, R1)
                        nc.vector.tensor_tensor(
                            asb, asb, mx[:, None, :].to_broadcast(bshp),
                            AluOp.subtract)
                        nc.scalar.activation(asb, asb, Act.Exp)
                        ssum = pas.tile([128, R1], F32, tag="ssum")
                        nc.vector.reduce_sum(ssum, ai, axis=mybir.AxisListType.X)
                        rcp = pas.tile([128, R1], F32, tag="rcp")
                        nc.vector.reciprocal(rcp, ssum)
                        wsr = pas.tile([128, 3, R1], BF16, tag="wsr")
                        nc.vector.tensor_tensor(
                            wsr, asb, rcp[:, None, :].to_broadcast(bshp),
                            AluOp.mult)

                        # ctx = sum_i ws_i * P_i -> dec_sb[:, 0:8]
                        shp = (128, 8, R1)
                        ctx = dec_sb[:, 0:8, :]
                        nc.vector.tensor_tensor(
                            ctx, wsr[:, 0, None, :].to_broadcast(shp),
                            pt[0], AluOp.mult)
                        t2 = pat.tile([128, 8, R1], BF16, tag="tmp", name="t2")
                        nc.vector.tensor_tensor(
                            t2, wsr[:, 1, None, :].to_broadcast(shp),
                            pt[1], AluOp.mult)
                        nc.vector.tensor_tensor(ctx, ctx, t2, AluOp.add)
                        t3 = pat.tile([128, 8, R1], BF16, tag="tmp", name="t3")
                        nc.vector.tensor_tensor(
                            t3, wsr[:, 2, None, :].to_broadcast(shp),
                            pt[2], AluOp.mult)
                        nc.vector.tensor_tensor(ctx, ctx, t3, AluOp.add)

                        nc.gpsimd.dma_start(dec[rt], dec_sb)

                        # xt prefetch emitted LAST: keeps its packets out
                        # of the slot-rotation while wh/earlier tiles are
                        # still streaming (needed ~60us later anyway).
                        if rt + 2 < NT1:
                            load_xt(rt + 2)

                # ================= Phase B =================
                with (
                    # pbd first: lands on the SBUF region freed earliest.
                    tc.tile_pool(name="pbd", bufs=2) as pbd,
                    tc.tile_pool(name="pbo", bufs=2) as pbo,
                    tc.tile_pool(name="pbe", bufs=3) as pbe,
                    tc.tile_pool(name="pbw", bufs=1) as pbw,
                    tc.tile_pool(name="psD", bufs=3, space="PSUM") as psD,
                    tc.tile_pool(name="psE", bufs=3, space="PSUM") as psE,
                ):
                    dcs = {}

                    def load_dc(rt):
                        t = pbd.tile([128, 16, R1], BF16, tag="dc", name="dc")
                        nc.sync.dma_start(t, dec[rt])
                        dcs[rt] = t

                    load_dc(0)
                    load_dc(1)
                    # wd2 on gpsimd: behind the dec stores in that FIFO,
                    # ready well before the first D2 needs it.
                    wd2_sb = pbw.tile([128, 16, OUT], BF16, tag="wd2")
                    for g in range(4):
                        nc.gpsimd.dma_start(wd2_sb[:, 4 * g:4 * g + 4, :],
                                            wd2T.ap()[:, 4 * g:4 * g + 4, :])

                    for rt in range(NT1):
                        rs = slice(rt * R1, (rt + 1) * R1)
                        if rt + 2 < NT1:
                            load_dc(rt + 2)
                        dc = dcs.pop(rt)

                        o1 = pbo.tile([128, 16, R1], BF16, tag="o1",
                                      name="o1")
                        for fc in range(16):
                            ps = psD.tile([128, R1], F32, tag="d1")
                            for kc in range(16):
                                nc.tensor.matmul(
                                    ps,
                                    wd1_sb[:, kc, fc * 128:(fc + 1) * 128],
                                    dc[:, kc, :],
                                    start=(kc == 0),
                                    stop=(kc == 15),
                                )
                            nc.scalar.activation(
                                o1[:, fc, :], ps, Act.Relu,
                                bias=bd1_sb[:, fc:fc + 1],
                            )
                        for oc in range(8):
                            ow = 128 if oc < 7 else OUT - 7 * 128
                            ps = psE.tile([128, R1], F32, tag="d2")
                            for kc in range(16):
                                nc.tensor.matmul(
                                    ps[:ow],
                                    wd2_sb[:, kc, oc * 128:oc * 128 + ow],
                                    o1[:, kc, :],
                                    start=(kc == 0),
                                    stop=(kc == 15),
                                )
                            ev = pbe.tile([128, R1], F32, tag="ev")
                            nc.vector.tensor_scalar_add(
                                ev[:ow], ps[:ow], bd2_sb[:ow, oc:oc + 1]
                            )
                            eng = nc.sync if oc % 2 else nc.gpsimd
                            eng.dma_start(
                                outT.ap()[oc * 128:oc * 128 + ow, rs],
                                ev[:ow],
                            )

    nc.finalize()
    return nc


def _prep_inputs(tube, w1_W, w1_b, w2_W, w2_b, w3_W, w3_b, wh_W, wh_b,
                 wd1_W, wd1_b, wd2_W, wd2_b):
    """Host-side reshape/transpose into the kernel's DRAM layouts."""
    f32 = np.float32
    bf16 = ml_dtypes.bfloat16

    def wT(w, kc):  # [F, K] -> partition-major [128, kc, F], bf16
        w = np.asarray(w, f32)
        return np.ascontiguousarray(
            w.T.astype(bf16).reshape(kc, 128, w.shape[0]).transpose(1, 0, 2))

    def bmat(b, cc):  # [F] -> [128, cc]
        b = np.asarray(b, f32)
        if b.shape[0] < cc * 128:
            b = np.pad(b, (0, cc * 128 - b.shape[0]))
        return np.ascontiguousarray(b.reshape(cc, 128).T)

    shared = {
        "w1T": wT(w1_W, 4), "w2T": wT(w2_W, 4), "w3T": wT(w3_W, 4),
        "whT": wT(wh_W, 24), "wd1T": wT(wd1_W, 16), "wd2T": wT(wd2_W, 16),
        "b1": bmat(w1_b, 8), "b2": bmat(w2_b, 8), "b3": bmat(w3_b, 8),
        "bh": bmat(wh_b, 8), "bd1": bmat(wd1_b, 16), "bd2": bmat(wd2_b, 8),
    }
    tubeT = np.ascontiguousarray(np.asarray(tube, f32).T.astype(bf16))
    in_maps = []
    for c in range(N_CORES):
        # [1536, ROWS] -> tiled partition-major [NT1, 128, 12, R1]
        xTc = np.ascontiguousarray(
            tubeT[:, c * ROWS:(c + 1) * ROWS]
            .reshape(12, 128, NT1, R1).transpose(2, 1, 0, 3))
        in_maps.append({"xT": xTc, **shared})
    return in_maps


_NC_CACHE = {}


def run(inputs, mm_dtype=None, trace=False):
    # mm_dtype kept for test.py compat; the kernel is bf16-only now.
    if "nc" not in _NC_CACHE:
        _NC_CACHE["nc"] = build_nc()
    nc = _NC_CACHE["nc"]
    in_maps = _prep_inputs(**inputs)
    res = run_bass_kernel_spmd(nc, in_maps, list(range(N_CORES)), trace=trace)
    out = np.empty((B, OUT), np.float32)
    for c in range(N_CORES):
        out[c * ROWS:(c + 1) * ROWS] = res.results[c]["outT"].T
    return out, res


def kernel(**inputs) -> np.ndarray:
    out, _ = run(inputs)
    return out
